# revision 56
# baseline (speedup 1.0000x reference)
"""Trainium2 Bass kernel for nn_Attention (dense transformer MHA block).

Reference computation (B=2, N=2048, D_MODEL=1024, H=16, D_K=D_V=64):
    q = (queries @ Wq.T)  -> (b, n, h, dk)   k, v likewise
    att = softmax(q k^T / sqrt(dk))
    out = queries + (att @ v) @ Wo.T + bo

Sharding over 8 NeuronCores: core c = (batch bi = c // 4) x (head-group
hg = c % 4, 4 heads each).  Tensor-parallel over heads: Wq/Wk/Wv split
column-wise (256 output features per core), Wo split row-wise; each core
produces a partial fc_o output (one core per batch also carries the
residual + bias) and the host sums the 4 partials per batch at gather
time (the "all-reduce" of the sharding hint, done on unshard).

Device dataflow per core (all matmuls bf16 with fp32 PSUM accumulate):
  - activations are fed pre-transposed (X.T layout: d_model on partitions)
  - q/k projections produce [feat, tok]; v projection produces [tok, feat]
    with a ones-column appended per head
  - scores computed transposed S_T[kt, qt] per head; exp on ScalarE with
    the 1/sqrt(dk) scale folded in; no max-subtraction (scores are O(1)
    by construction, exp is safe in fp32)
  - att @ v accumulates over kt tiles in PSUM; the ones-column yields the
    softmax denominator for free; normalization applied once on the
    [64, qt] av output, not on the [2048, qt] att matrix
  - fc_o in [e, qt] orientation; bias enters via a K=1 matmul against a
    ones row; residual added from the resident qT tiles gated by a
    per-core flag input
"""

import os
import sys
import types

import ml_dtypes
import numpy as np

_TRN_REPO = "/opt/trn_rl_repo"
if _TRN_REPO not in sys.path:
    sys.path.insert(0, _TRN_REPO)


def _install_ntff_hook():
    """Make run_bass_kernel_spmd(trace=True) work under axon: the agent
    image's antenv lacks axon_hooks, so synthesize it from the boot
    helper. Harmless if tracing is never requested."""
    if "antenv.axon_hooks" in sys.modules:
        return
    try:
        from trn_agent_boot.trn_boot import _ntff_profile_via_ctypes

        mod = types.ModuleType("antenv.axon_hooks")
        hook = _ntff_profile_via_ctypes("/opt/axon/libaxon_pjrt.so")
        mod.get_axon_ntff_profile_hook = lambda: hook
        mod.set_axon_ntff_profile_hook = lambda h: None
        sys.modules["antenv.axon_hooks"] = mod
    except Exception:
        pass


_install_ntff_hook()

import concourse.bass as bass  # noqa: E402
import concourse.mybir as mybir  # noqa: E402
import concourse.tile as tile  # noqa: E402
from concourse import bacc  # noqa: E402
import concourse.bass_utils as bass_utils  # noqa: E402

# No artifact bucket in this container; tracing only needs the local files.
bass_utils.upload_artifacts = lambda tmpdir: ""



F32 = mybir.dt.float32
BF16 = mybir.dt.bfloat16

B, N, DM, H, DK = 2, 2048, 1024, 16, 64
NCORES = 8
HG = 4            # head-groups (tensor-parallel degree per batch)
NH = H // HG      # heads per core = 4
F = NH * DK       # projected features per core = 256
P = 128
ND = DM // P      # d_model k-tiles = 8
NKT = N // P      # key tiles = 16
QS = 512          # qt stripe for matmul N
NQS = N // QS     # = 4
SCALE = 1.0 / np.sqrt(DK)


def build_bass():
    nc = bacc.Bacc("TRN2", target_bir_lowering=False, debug=False,
                   num_devices=NCORES, num_swdge_queues=1)

    def din(name, shape, dt=F32):
        return nc.dram_tensor(name, list(shape), dt, kind="ExternalInput").ap()

    qT_d = din("qT", (DM, N))
    kT_d = din("kT", (DM, N))
    vT_d = din("vT", (DM, N))
    wq_d = din("wq", (DM, F), BF16)
    wk_d = din("wk", (DM, F), BF16)
    wv_d = din("wv", (DM, F), BF16)
    wo_d = din("wo", (F, DM), BF16)
    rfl_d = din("resflag", (P, 1))
    out_d = nc.dram_tensor("out", [DM, N], F32, kind="ExternalOutput").ap()

    qT_r = qT_d.rearrange("(a p) t -> p a t", p=P)
    kT_r = kT_d.rearrange("(a p) t -> p a t", p=P)
    vT_r = vT_d.rearrange("(a p) t -> p a t", p=P)
    wq_r = wq_d.rearrange("(a p) f -> p a f", p=P)
    wk_r = wk_d.rearrange("(a p) f -> p a f", p=P)
    wv_r = wv_d.rearrange("(a p) f -> p a f", p=P)
    wo_r = wo_d.rearrange("(a p) e -> p a e", p=P)
    out_r = out_d.rearrange("(a p) t -> p a t", p=P)

    with tile.TileContext(nc) as tc:
        with (
            tc.tile_pool(name="wpool", bufs=1) as wpool,
            tc.tile_pool(name="xq", bufs=1) as xq,
            tc.tile_pool(name="xk", bufs=1) as xk,
            tc.tile_pool(name="xv", bufs=1) as xv,
            tc.tile_pool(name="qk", bufs=1) as qkp,
            tc.tile_pool(name="vsb", bufs=1) as vsbp,
            tc.tile_pool(name="aop", bufs=1) as aop,
            tc.tile_pool(name="attp", bufs=3) as attp,
            tc.tile_pool(name="smallp", bufs=3) as smallp,
            tc.tile_pool(name="outp", bufs=2) as outp,
            tc.tile_pool(name="pp", bufs=2, space="PSUM") as pp,
            tc.tile_pool(name="pss", bufs=2, space="PSUM") as pss,
            tc.tile_pool(name="pav", bufs=2, space="PSUM") as pav,
        ):
            # ---- persistent SBUF tensors
            wq_bf = wpool.tile([P, ND, F], BF16)
            wk_bf = wpool.tile([P, ND, F], BF16)
            wv_bf = wpool.tile([P, ND, F], BF16)
            wo_bf = wpool.tile([P, F // P, DM], BF16)
            rfl_sb = wpool.tile([P, 1], F32)
            qT_bf = xq.tile([P, ND, N], BF16)
            kT_bf = xk.tile([P, ND, N], BF16)
            vT_bf = xv.tile([P, ND, N], BF16)
            q_sb = qkp.tile([P, F // P, N], BF16)
            k_sb = qkp.tile([P, F // P, N], BF16)
            v_sb = vsbp.tile([P, NKT, NH, DK + 1], BF16)
            attout = aop.tile([P, F // P, N], BF16)

            # ---- input DMAs (SWDGE: fp32 DRAM -> bf16 SBUF cast), ordered
            # so the attention-critical tensors (full kT, first qT stripe)
            # land first; weights interleave just before their projection
            nc.sync.dma_start(out=rfl_sb[:, :], in_=rfl_d[:, :])
            nc.vector.memset(v_sb[:, :, :, :], 1.0)

            # Feed in token-block-major chunks (all d-tiles per chunk, so
            # each chunk unlocks its consumers immediately), interleaved to
            # match the first attention unit's consumption order.
            def chunk(dst, src, t0, t1):
                nc.gpsimd.dma_start(out=dst[:, :, t0:t1], in_=src[:, :, t0:t1])

            nc.gpsimd.dma_start(out=wk_bf[:, :, :], in_=wk_r[:, :, :])
            chunk(kT_bf, kT_r, 0, QS)
            nc.gpsimd.dma_start(out=wq_bf[:, :, :], in_=wq_r[:, :, :])
            chunk(qT_bf, qT_r, 0, QS)
            nc.gpsimd.dma_start(out=wv_bf[:, :, :], in_=wv_r[:, :, :])
            chunk(vT_bf, vT_r, 0, 2 * P)
            chunk(kT_bf, kT_r, QS, 2 * QS)
            chunk(vT_bf, vT_r, 2 * P, 4 * P)
            chunk(kT_bf, kT_r, 2 * QS, 3 * QS)
            chunk(vT_bf, vT_r, 4 * P, 6 * P)
            chunk(kT_bf, kT_r, 3 * QS, 4 * QS)
            chunk(vT_bf, vT_r, 6 * P, 8 * P)
            chunk(vT_bf, vT_r, 8 * P, 12 * P)
            chunk(vT_bf, vT_r, 12 * P, 16 * P)
            chunk(qT_bf, qT_r, QS, 2 * QS)
            chunk(qT_bf, qT_r, 2 * QS, 3 * QS)
            chunk(qT_bf, qT_r, 3 * QS, 4 * QS)
            nc.gpsimd.dma_start(out=wo_bf[:, :, :], in_=wo_r[:, :, :])

            # ---- projections, first token-half of k/q/v, then second half
            def kq_proj(w_bf, x_bf, dst, ts):
                for ft in range(F // P):
                    ps = pp.tile([P, QS], F32, tag="pp", name="ps_kq")
                    for a in range(ND):
                        nc.tensor.matmul(
                            ps[:, :],
                            lhsT=w_bf[:, a, ft * P:(ft + 1) * P],
                            rhs=x_bf[:, a, ts * QS:(ts + 1) * QS],
                            start=(a == 0), stop=(a == ND - 1),
                        )
                    nc.vector.tensor_copy(dst[:, ft, ts * QS:(ts + 1) * QS],
                                          ps[:, :])

            def v_proj(kt):
                ps = pp.tile([P, F], F32, tag="pp", name="ps_v")
                for a in range(ND):
                    nc.tensor.matmul(
                        ps[:, :],
                        lhsT=vT_bf[:, a, kt * P:(kt + 1) * P],
                        rhs=wv_bf[:, a, :],
                        start=(a == 0), stop=(a == ND - 1),
                    )
                nc.vector.tensor_copy(
                    v_sb[:, kt, :, 0:DK],
                    ps[:, :].rearrange("p (h d) -> p h d", h=NH),
                )

            kq_proj(wk_bf, kT_bf, k_sb, 0)
            kq_proj(wq_bf, qT_bf, q_sb, 0)

            # fc_o for one stripe, emitted in e-tile chunks so it spreads
            # across the next stripe's attention units instead of clumping
            # (each engine's instruction order is static; a clump at the
            # stripe boundary starves ScalarE of exp work for ~15us)
            def fc_o(qs, out_sb, a_lo, a_hi):
                q0 = qs * QS
                for a in range(a_lo, a_hi):
                    ps_o = pp.tile([P, QS], F32, tag="pp", name=f"o_{qs}_{a}")
                    for ht in range(F // P):
                        nc.tensor.matmul(
                            ps_o[:, :],
                            lhsT=wo_bf[:, ht, a * P:(a + 1) * P],
                            rhs=attout[:, ht, q0:q0 + QS],
                            start=(ht == 0), stop=(ht == F // P - 1),
                        )
                    nc.vector.scalar_tensor_tensor(
                        out=out_sb[:, a, :],
                        in0=qT_bf[:, a, q0:q0 + QS],
                        scalar=rfl_sb[:, 0:1],
                        in1=ps_o[:, :],
                        op0=mybir.AluOpType.mult,
                        op1=mybir.AluOpType.add,
                    )
                if a_hi == ND:
                    nc.sync.dma_start(out=out_r[:, :, q0:q0 + QS],
                                      in_=out_sb[:, :, :])

            # ---- attention: unit = (qs stripe, head-PAIR hp).  Per kt tile
            # the two heads of an f-tile interleave their score MMs (h-even
            # in rows 0:64, h-odd in rows 64:128) so each k LDWEIGHTS hides
            # under the other head's in-flight matmul; one [128, 1024] exp
            # covers both heads; two av accumulators run in parallel and are
            # copied to SBUF at unit end so the PSUM slots rotate immediately.
            def normalize(av_cp, dcol, hp, i, q0):
                po, ft = DK * i, hp
                recip = smallp.tile([1, QS], F32, tag="recip")
                # approx_fast (51 ULP) is plenty, but this custom-DVE op
                # needs an SBUF source at base partition 0 (dcol).
                nc.vector.reciprocal_approx_fast(recip[:, :], dcol[:, :])
                recipb = smallp.tile([DK, QS], F32, tag="recipb")
                nc.gpsimd.partition_broadcast(recipb[:, :], recip[:, :])
                nc.vector.tensor_mul(
                    attout[po:po + DK, ft, q0:q0 + QS],
                    av_cp[0:DK, :],
                    recipb[:, :],
                )

            prev_out_sb = None
            for qs in range(NQS):
                q0 = qs * QS
                cur_out_sb = outp.tile([P, ND, QS], F32, tag="osb",
                                       name=f"osb_{qs}")
                for hp in range(2):
                    if qs == NQS - 1 and hp == 1:
                        # final stripe: the f-tile-0 half of fc_o overlaps
                        # the second attention unit to shorten the tail
                        for a in range(ND):
                            ps_o = pp.tile([P, QS], F32, tag="pp",
                                           name=f"ox_{a}")
                            nc.tensor.matmul(
                                ps_o[:, :],
                                lhsT=wo_bf[:, 0, a * P:(a + 1) * P],
                                rhs=attout[:, 0, q0:q0 + QS],
                                start=True, stop=True,
                            )
                            nc.vector.scalar_tensor_tensor(
                                out=cur_out_sb[:, a, :],
                                in0=qT_bf[:, a, q0:q0 + QS],
                                scalar=rfl_sb[:, 0:1],
                                in1=ps_o[:, :],
                                op0=mybir.AluOpType.mult,
                                op1=mybir.AluOpType.add,
                            )
                    if qs < NQS - 1 and hp == 1:
                        kq_proj(wq_bf, qT_bf, q_sb, qs + 1)  # prefetch q-proj
                    ps_av = [pav.tile([DK + 1, QS], F32, tag="pav",
                                      name=f"av_{qs}_{hp}_{i}")
                             for i in range(2)]

                    for kt in range(NKT):
                        if qs == 0 and hp == 0:
                            v_proj(kt)  # v-proj rides just ahead of use
                            if kt in (2, 6, 10) and kt // 4 + 1 < NQS:
                                kq_proj(wk_bf, kT_bf, k_sb, kt // 4 + 1)
                        if qs > 0 and kt in (1, 3, 5, 7):
                            # previous stripe's fc_o woven into the early kt
                            # slots (ACT paces here, PE has slack)
                            a = 4 * hp + (kt - 1) // 2
                            fc_o(qs - 1, prev_out_sb, a, a + 1)
                        ps_s = pss.tile([P, 2 * QS], F32, tag="pss")
                        for i in range(2):
                            po = DK * i
                            nc.tensor.matmul(
                                ps_s[:, i * QS:(i + 1) * QS],
                                lhsT=k_sb[po:po + DK, hp, kt * P:(kt + 1) * P],
                                rhs=q_sb[po:po + DK, hp, q0:q0 + QS],
                                start=True, stop=True,
                            )
                        att = attp.tile([P, 2 * QS], BF16, tag="att")
                        nc.scalar.activation(att[:, :], ps_s[:, :],
                                             mybir.ActivationFunctionType.Exp,
                                             scale=float(SCALE))
                        for i in range(2):
                            nc.tensor.matmul(
                                ps_av[i][:, :],
                                lhsT=v_sb[:, kt, 2 * hp + i, :],
                                rhs=att[:, i * QS:(i + 1) * QS],
                                start=(kt == 0), stop=(kt == NKT - 1),
                            )
                    for i in range(2):
                        av_cp = smallp.tile([DK, QS], F32, tag="avcp")
                        nc.vector.tensor_copy(av_cp[:, :], ps_av[i][0:DK, :])
                        dcol = smallp.tile([1, QS], F32, tag="dcol")
                        nc.vector.tensor_copy(dcol[:, :],
                                              ps_av[i][DK:DK + 1, :])
                        normalize(av_cp, dcol, hp, i, q0)
                if qs == NQS - 1:
                    # hd-tile-1 half of fc_o, accumulated onto the pass-1 rows
                    for a in range(ND):
                        ps_o = pp.tile([P, QS], F32, tag="pp", name=f"o2_{a}")
                        nc.tensor.matmul(
                            ps_o[:, :],
                            lhsT=wo_bf[:, 1, a * P:(a + 1) * P],
                            rhs=attout[:, 1, q0:q0 + QS],
                            start=True, stop=True,
                        )
                        nc.vector.tensor_add(cur_out_sb[:, a, :],
                                             ps_o[:, :],
                                             cur_out_sb[:, a, :])
                        if a % 2 == 1:  # stream the tail out in 2-tile chunks
                            nc.sync.dma_start(
                                out=out_r[:, a - 1:a + 1, q0:q0 + QS],
                                in_=cur_out_sb[:, a - 1:a + 1, :])
                prev_out_sb = cur_out_sb

    nc.compile()
    return nc


_NC_CACHE = None


def _get_nc():
    global _NC_CACHE
    if _NC_CACHE is None:
        _NC_CACHE = build_bass()
    return _NC_CACHE


def kernel(queries, keys, values, Wq, Wk, Wv, Wo, bo):
    queries = np.asarray(queries, dtype=np.float32)
    keys = np.asarray(keys, dtype=np.float32)
    values = np.asarray(values, dtype=np.float32)
    Wq = np.asarray(Wq, dtype=np.float32)
    Wk = np.asarray(Wk, dtype=np.float32)
    Wv = np.asarray(Wv, dtype=np.float32)
    Wo = np.asarray(Wo, dtype=np.float32)
    bo = np.asarray(bo, dtype=np.float32)

    nc = _get_nc()

    in_maps = []
    for c in range(NCORES):
        bi, hg = c // HG, c % HG
        sl = slice(hg * F, (hg + 1) * F)
        bf = ml_dtypes.bfloat16  # static weights pre-cast at shard time
        in_maps.append({
            "qT": np.ascontiguousarray(queries[bi].T),
            "kT": np.ascontiguousarray(keys[bi].T),
            "vT": np.ascontiguousarray(values[bi].T),
            "wq": np.ascontiguousarray(Wq[sl, :].T).astype(bf),
            "wk": np.ascontiguousarray(Wk[sl, :].T).astype(bf),
            "wv": np.ascontiguousarray(Wv[sl, :].T).astype(bf),
            "wo": np.ascontiguousarray(Wo[:, sl].T).astype(bf),
            "resflag": np.full((P, 1), 1.0 if hg == 0 else 0.0,
                               dtype=np.float32),
        })

    trace = bool(os.environ.get("BASS_TRACE"))
    res = bass_utils.run_bass_kernel_spmd(
        nc, in_maps, core_ids=list(range(NCORES)), trace=trace)
    kernel.last_exec_time_ns = res.exec_time_ns

    outs = [res.results[c]["out"] for c in range(NCORES)]
    full = np.stack([
        (outs[0] + outs[1] + outs[2] + outs[3]).T,
        (outs[4] + outs[5] + outs[6] + outs[7]).T,
    ]).astype(np.float32)
    full += bo  # unshard epilogue: bias is a per-batch additive constant
    return full


# revision 57
# speedup vs baseline: 1.0110x; 1.0110x over previous
"""Trainium2 Bass kernel for nn_Attention (dense transformer MHA block).

Reference computation (B=2, N=2048, D_MODEL=1024, H=16, D_K=D_V=64):
    q = (queries @ Wq.T)  -> (b, n, h, dk)   k, v likewise
    att = softmax(q k^T / sqrt(dk))
    out = queries + (att @ v) @ Wo.T + bo

Sharding over 8 NeuronCores: core c = (batch bi = c // 4) x (head-group
hg = c % 4, 4 heads each).  Tensor-parallel over heads: Wq/Wk/Wv split
column-wise (256 output features per core), Wo split row-wise; each core
produces a partial fc_o output (one core per batch also carries the
residual + bias) and the host sums the 4 partials per batch at gather
time (the "all-reduce" of the sharding hint, done on unshard).

Device dataflow per core (all matmuls bf16 with fp32 PSUM accumulate):
  - activations are fed pre-transposed (X.T layout: d_model on partitions)
  - q/k projections produce [feat, tok]; v projection produces [tok, feat]
    with a ones-column appended per head
  - scores computed transposed S_T[kt, qt] per head; exp on ScalarE with
    the 1/sqrt(dk) scale folded in; no max-subtraction (scores are O(1)
    by construction, exp is safe in fp32)
  - att @ v accumulates over kt tiles in PSUM; the ones-column yields the
    softmax denominator for free; normalization applied once on the
    [64, qt] av output, not on the [2048, qt] att matrix
  - fc_o in [e, qt] orientation; bias enters via a K=1 matmul against a
    ones row; residual added from the resident qT tiles gated by a
    per-core flag input
"""

import os
import sys
import types

import ml_dtypes
import numpy as np

_TRN_REPO = "/opt/trn_rl_repo"
if _TRN_REPO not in sys.path:
    sys.path.insert(0, _TRN_REPO)


def _install_ntff_hook():
    """Make run_bass_kernel_spmd(trace=True) work under axon: the agent
    image's antenv lacks axon_hooks, so synthesize it from the boot
    helper. Harmless if tracing is never requested."""
    if "antenv.axon_hooks" in sys.modules:
        return
    try:
        from trn_agent_boot.trn_boot import _ntff_profile_via_ctypes

        mod = types.ModuleType("antenv.axon_hooks")
        hook = _ntff_profile_via_ctypes("/opt/axon/libaxon_pjrt.so")
        mod.get_axon_ntff_profile_hook = lambda: hook
        mod.set_axon_ntff_profile_hook = lambda h: None
        sys.modules["antenv.axon_hooks"] = mod
    except Exception:
        pass


_install_ntff_hook()

import concourse.bass as bass  # noqa: E402
import concourse.mybir as mybir  # noqa: E402
import concourse.tile as tile  # noqa: E402
from concourse import bacc  # noqa: E402
import concourse.bass_utils as bass_utils  # noqa: E402

# No artifact bucket in this container; tracing only needs the local files.
bass_utils.upload_artifacts = lambda tmpdir: ""



F32 = mybir.dt.float32
BF16 = mybir.dt.bfloat16

B, N, DM, H, DK = 2, 2048, 1024, 16, 64
NCORES = 8
HG = 4            # head-groups (tensor-parallel degree per batch)
NH = H // HG      # heads per core = 4
F = NH * DK       # projected features per core = 256
P = 128
ND = DM // P      # d_model k-tiles = 8
NKT = N // P      # key tiles = 16
QS = 512          # qt stripe for matmul N
NQS = N // QS     # = 4
SCALE = 1.0 / np.sqrt(DK)


def build_bass():
    nc = bacc.Bacc("TRN2", target_bir_lowering=False, debug=False,
                   num_devices=NCORES, num_swdge_queues=1)

    def din(name, shape, dt=F32):
        return nc.dram_tensor(name, list(shape), dt, kind="ExternalInput").ap()

    qT_d = din("qT", (DM, N))
    kT_d = din("kT", (DM, N))
    vT_d = din("vT", (DM, N))
    wq_d = din("wq", (DM, F), BF16)
    wk_d = din("wk", (DM, F), BF16)
    wv_d = din("wv", (DM, F), BF16)
    wo_d = din("wo", (F, DM), BF16)
    rfl_d = din("resflag", (P, 1))
    out_d = nc.dram_tensor("out", [DM, N], F32, kind="ExternalOutput").ap()

    qT_r = qT_d.rearrange("(a p) t -> p a t", p=P)
    kT_r = kT_d.rearrange("(a p) t -> p a t", p=P)
    vT_r = vT_d.rearrange("(a p) t -> p a t", p=P)
    wq_r = wq_d.rearrange("(a p) f -> p a f", p=P)
    wk_r = wk_d.rearrange("(a p) f -> p a f", p=P)
    wv_r = wv_d.rearrange("(a p) f -> p a f", p=P)
    wo_r = wo_d.rearrange("(a p) e -> p a e", p=P)
    out_r = out_d.rearrange("(a p) t -> p a t", p=P)

    with tile.TileContext(nc) as tc:
        with (
            tc.tile_pool(name="wpool", bufs=1) as wpool,
            tc.tile_pool(name="xq", bufs=1) as xq,
            tc.tile_pool(name="xk", bufs=1) as xk,
            tc.tile_pool(name="xv", bufs=1) as xv,
            tc.tile_pool(name="qk", bufs=1) as qkp,
            tc.tile_pool(name="vsb", bufs=1) as vsbp,
            tc.tile_pool(name="aop", bufs=1) as aop,
            tc.tile_pool(name="attp", bufs=3) as attp,
            tc.tile_pool(name="smallp", bufs=2) as smallp,
            tc.tile_pool(name="outp", bufs=2) as outp,
            tc.tile_pool(name="pp", bufs=2, space="PSUM") as pp,
            tc.tile_pool(name="pss", bufs=2, space="PSUM") as pss,
            tc.tile_pool(name="pav", bufs=2, space="PSUM") as pav,
        ):
            # ---- persistent SBUF tensors
            wq_bf = wpool.tile([P, ND, F], BF16)
            wk_bf = wpool.tile([P, ND, F], BF16)
            wv_bf = wpool.tile([P, ND, F], BF16)
            wo_bf = wpool.tile([P, F // P, DM], BF16)
            rfl_sb = wpool.tile([P, 1], F32)
            qT_bf = xq.tile([P, ND, N], BF16)
            kT_bf = xk.tile([P, ND, N], BF16)
            vT_bf = xv.tile([P, ND, N], BF16)
            q_sb = qkp.tile([P, F // P, N], BF16)
            k_sb = qkp.tile([P, F // P, N], BF16)
            v_sb = vsbp.tile([P, NKT, NH, DK + 1], BF16)
            attout = aop.tile([P, F // P, N], BF16)

            # ---- input DMAs (SWDGE: fp32 DRAM -> bf16 SBUF cast), ordered
            # so the attention-critical tensors (full kT, first qT stripe)
            # land first; weights interleave just before their projection
            nc.sync.dma_start(out=rfl_sb[:, :], in_=rfl_d[:, :])
            nc.vector.memset(v_sb[:, :, :, :], 1.0)

            # Feed in token-block-major chunks (all d-tiles per chunk, so
            # each chunk unlocks its consumers immediately), interleaved to
            # match the first attention unit's consumption order.
            def chunk(dst, src, t0, t1):
                nc.gpsimd.dma_start(out=dst[:, :, t0:t1], in_=src[:, :, t0:t1])

            nc.gpsimd.dma_start(out=wk_bf[:, :, :], in_=wk_r[:, :, :])
            chunk(kT_bf, kT_r, 0, QS)
            nc.gpsimd.dma_start(out=wq_bf[:, :, :], in_=wq_r[:, :, :])
            chunk(qT_bf, qT_r, 0, QS)
            nc.gpsimd.dma_start(out=wv_bf[:, :, :], in_=wv_r[:, :, :])
            chunk(vT_bf, vT_r, 0, 2 * P)
            chunk(kT_bf, kT_r, QS, 2 * QS)
            chunk(vT_bf, vT_r, 2 * P, 4 * P)
            chunk(kT_bf, kT_r, 2 * QS, 3 * QS)
            chunk(vT_bf, vT_r, 4 * P, 6 * P)
            chunk(kT_bf, kT_r, 3 * QS, 4 * QS)
            chunk(vT_bf, vT_r, 6 * P, 8 * P)
            chunk(vT_bf, vT_r, 8 * P, 12 * P)
            chunk(vT_bf, vT_r, 12 * P, 16 * P)
            chunk(qT_bf, qT_r, QS, 2 * QS)
            chunk(qT_bf, qT_r, 2 * QS, 3 * QS)
            chunk(qT_bf, qT_r, 3 * QS, 4 * QS)
            nc.gpsimd.dma_start(out=wo_bf[:, :, :], in_=wo_r[:, :, :])

            # ---- projections, first token-half of k/q/v, then second half
            def kq_proj(w_bf, x_bf, dst, ts):
                for ft in range(F // P):
                    ps = pp.tile([P, QS], F32, tag="pp", name="ps_kq")
                    for a in range(ND):
                        nc.tensor.matmul(
                            ps[:, :],
                            lhsT=w_bf[:, a, ft * P:(ft + 1) * P],
                            rhs=x_bf[:, a, ts * QS:(ts + 1) * QS],
                            start=(a == 0), stop=(a == ND - 1),
                        )
                    nc.vector.tensor_copy(dst[:, ft, ts * QS:(ts + 1) * QS],
                                          ps[:, :])

            def v_proj(kt):
                ps = pp.tile([P, F], F32, tag="pp", name="ps_v")
                for a in range(ND):
                    nc.tensor.matmul(
                        ps[:, :],
                        lhsT=vT_bf[:, a, kt * P:(kt + 1) * P],
                        rhs=wv_bf[:, a, :],
                        start=(a == 0), stop=(a == ND - 1),
                    )
                nc.vector.tensor_copy(
                    v_sb[:, kt, :, 0:DK],
                    ps[:, :].rearrange("p (h d) -> p h d", h=NH),
                )

            kq_proj(wk_bf, kT_bf, k_sb, 0)
            kq_proj(wq_bf, qT_bf, q_sb, 0)

            # fc_o for one stripe, emitted in e-tile chunks so it spreads
            # across the next stripe's attention units instead of clumping
            # (each engine's instruction order is static; a clump at the
            # stripe boundary starves ScalarE of exp work for ~15us)
            def fc_o(qs, out_sb, a_lo, a_hi):
                q0 = qs * QS
                for a in range(a_lo, a_hi):
                    ps_o = pp.tile([P, QS], F32, tag="pp", name=f"o_{qs}_{a}")
                    for ht in range(F // P):
                        nc.tensor.matmul(
                            ps_o[:, :],
                            lhsT=wo_bf[:, ht, a * P:(a + 1) * P],
                            rhs=attout[:, ht, q0:q0 + QS],
                            start=(ht == 0), stop=(ht == F // P - 1),
                        )
                    nc.vector.scalar_tensor_tensor(
                        out=out_sb[:, a, :],
                        in0=qT_bf[:, a, q0:q0 + QS],
                        scalar=rfl_sb[:, 0:1],
                        in1=ps_o[:, :],
                        op0=mybir.AluOpType.mult,
                        op1=mybir.AluOpType.add,
                    )
                if a_hi == ND:
                    nc.sync.dma_start(out=out_r[:, :, q0:q0 + QS],
                                      in_=out_sb[:, :, :])

            # ---- attention: unit = (qs stripe, head-PAIR hp).  Per kt tile
            # the two heads of an f-tile interleave their score MMs (h-even
            # in rows 0:64, h-odd in rows 64:128) so each k LDWEIGHTS hides
            # under the other head's in-flight matmul; one [128, 1024] exp
            # covers both heads; two av accumulators run in parallel and are
            # copied to SBUF at unit end so the PSUM slots rotate immediately.
            def normalize(av_cp, dcol, hp, i, q0):
                po, ft = DK * i, hp
                recip = smallp.tile([1, QS], F32, tag="recip")
                # approx_fast (51 ULP) is plenty, but this custom-DVE op
                # needs an SBUF source at base partition 0 (dcol).
                nc.vector.reciprocal_approx_fast(recip[:, :], dcol[:, :])
                recipb = smallp.tile([DK, QS], F32, tag="recipb")
                nc.gpsimd.partition_broadcast(recipb[:, :], recip[:, :])
                nc.vector.tensor_mul(
                    attout[po:po + DK, ft, q0:q0 + QS],
                    av_cp[0:DK, :],
                    recipb[:, :],
                )

            prev_out_sb = None
            for qs in range(NQS):
                q0 = qs * QS
                cur_out_sb = outp.tile([P, ND, QS], F32, tag="osb",
                                       name=f"osb_{qs}")
                for hp in range(2):
                    if qs == NQS - 1 and hp == 1:
                        # final stripe: the f-tile-0 half of fc_o overlaps
                        # the second attention unit to shorten the tail
                        for a in range(ND):
                            ps_o = pp.tile([P, QS], F32, tag="pp",
                                           name=f"ox_{a}")
                            nc.tensor.matmul(
                                ps_o[:, :],
                                lhsT=wo_bf[:, 0, a * P:(a + 1) * P],
                                rhs=attout[:, 0, q0:q0 + QS],
                                start=True, stop=True,
                            )
                            nc.vector.scalar_tensor_tensor(
                                out=cur_out_sb[:, a, :],
                                in0=qT_bf[:, a, q0:q0 + QS],
                                scalar=rfl_sb[:, 0:1],
                                in1=ps_o[:, :],
                                op0=mybir.AluOpType.mult,
                                op1=mybir.AluOpType.add,
                            )
                    if qs < NQS - 1 and hp == 1:
                        kq_proj(wq_bf, qT_bf, q_sb, qs + 1)  # prefetch q-proj
                    ps_av = [pav.tile([DK + 1, QS], F32, tag="pav",
                                      name=f"av_{qs}_{hp}_{i}")
                             for i in range(2)]

                    for kt in range(NKT):
                        if qs == 0 and hp == 0:
                            v_proj(kt)  # v-proj rides just ahead of use
                            if kt in (2, 6, 10) and kt // 4 + 1 < NQS:
                                kq_proj(wk_bf, kT_bf, k_sb, kt // 4 + 1)
                        if qs > 0 and kt in (1, 3, 5, 7):
                            # previous stripe's fc_o woven into the early kt
                            # slots (ACT paces here, PE has slack)
                            a = 4 * hp + (kt - 1) // 2
                            fc_o(qs - 1, prev_out_sb, a, a + 1)
                        ps_s = pss.tile([P, 2 * QS], F32, tag="pss")
                        for i in range(2):
                            po = DK * i
                            nc.tensor.matmul(
                                ps_s[:, i * QS:(i + 1) * QS],
                                lhsT=k_sb[po:po + DK, hp, kt * P:(kt + 1) * P],
                                rhs=q_sb[po:po + DK, hp, q0:q0 + QS],
                                start=True, stop=True,
                            )
                        att = attp.tile([P, 2 * QS], BF16, tag="att")
                        nc.scalar.activation(att[:, :], ps_s[:, :],
                                             mybir.ActivationFunctionType.Exp,
                                             scale=float(SCALE))
                        for i in range(2):
                            nc.tensor.matmul(
                                ps_av[i][:, :],
                                lhsT=v_sb[:, kt, 2 * hp + i, :],
                                rhs=att[:, i * QS:(i + 1) * QS],
                                start=(kt == 0), stop=(kt == NKT - 1),
                            )
                    for i in range(2):
                        av_cp = smallp.tile([DK, QS], F32, tag="avcp")
                        nc.vector.tensor_copy(av_cp[:, :], ps_av[i][0:DK, :])
                        dcol = smallp.tile([1, QS], F32, tag="dcol")
                        nc.vector.tensor_copy(dcol[:, :],
                                              ps_av[i][DK:DK + 1, :])
                        normalize(av_cp, dcol, hp, i, q0)
                if qs == NQS - 1:
                    # hd-tile-1 half of fc_o, accumulated onto the pass-1 rows
                    for a in range(ND):
                        ps_o = pp.tile([P, QS], F32, tag="pp", name=f"o2_{a}")
                        nc.tensor.matmul(
                            ps_o[:, :],
                            lhsT=wo_bf[:, 1, a * P:(a + 1) * P],
                            rhs=attout[:, 1, q0:q0 + QS],
                            start=True, stop=True,
                        )
                        nc.vector.tensor_add(cur_out_sb[:, a, :],
                                             ps_o[:, :],
                                             cur_out_sb[:, a, :])
                        if a % 2 == 1:  # stream the tail out in 2-tile chunks
                            nc.sync.dma_start(
                                out=out_r[:, a - 1:a + 1, q0:q0 + QS],
                                in_=cur_out_sb[:, a - 1:a + 1, :])
                prev_out_sb = cur_out_sb

    nc.compile()
    return nc


_NC_CACHE = None


def _get_nc():
    global _NC_CACHE
    if _NC_CACHE is None:
        _NC_CACHE = build_bass()
    return _NC_CACHE


def kernel(queries, keys, values, Wq, Wk, Wv, Wo, bo):
    queries = np.asarray(queries, dtype=np.float32)
    keys = np.asarray(keys, dtype=np.float32)
    values = np.asarray(values, dtype=np.float32)
    Wq = np.asarray(Wq, dtype=np.float32)
    Wk = np.asarray(Wk, dtype=np.float32)
    Wv = np.asarray(Wv, dtype=np.float32)
    Wo = np.asarray(Wo, dtype=np.float32)
    bo = np.asarray(bo, dtype=np.float32)

    nc = _get_nc()

    in_maps = []
    for c in range(NCORES):
        bi, hg = c // HG, c % HG
        sl = slice(hg * F, (hg + 1) * F)
        bf = ml_dtypes.bfloat16  # static weights pre-cast at shard time
        in_maps.append({
            "qT": np.ascontiguousarray(queries[bi].T),
            "kT": np.ascontiguousarray(keys[bi].T),
            "vT": np.ascontiguousarray(values[bi].T),
            "wq": np.ascontiguousarray(Wq[sl, :].T).astype(bf),
            "wk": np.ascontiguousarray(Wk[sl, :].T).astype(bf),
            "wv": np.ascontiguousarray(Wv[sl, :].T).astype(bf),
            "wo": np.ascontiguousarray(Wo[:, sl].T).astype(bf),
            "resflag": np.full((P, 1), 1.0 if hg == 0 else 0.0,
                               dtype=np.float32),
        })

    trace = bool(os.environ.get("BASS_TRACE"))
    res = bass_utils.run_bass_kernel_spmd(
        nc, in_maps, core_ids=list(range(NCORES)), trace=trace)
    kernel.last_exec_time_ns = res.exec_time_ns

    outs = [res.results[c]["out"] for c in range(NCORES)]
    full = np.stack([
        (outs[0] + outs[1] + outs[2] + outs[3]).T,
        (outs[4] + outs[5] + outs[6] + outs[7]).T,
    ]).astype(np.float32)
    full += bo  # unshard epilogue: bias is a per-batch additive constant
    return full
